# revision 24
# baseline (speedup 1.0000x reference)
"""Trainium2 Bass kernel for DigitConvolutionalModel (conv3x3 -> FC512 -> FC10).

Math: the 3x3 valid conv is linear, so y_flat = x @ C with C [784, 676]
banded; W1_eff = C @ W1 is folded on the HOST (9 terms per row), so the
device runs only
    logits = relu(x @ W1_eff + b1) @ W2 + b2
Data-parallel across 8 cores on the batch dim (2048 rows per core).

Device schedule per core:
  - warmup matmuls ramp the PE while the first chunks stream in
  - supply (w1eff chunks, x per (superblock, t)) triggered round-robin
    over the three DMA rings in exact consumption order; per-transfer
    completion latency is ~1.5-3.5us under load, aggregate ~240GB/s
  - L1 superblocks 0-2 run t-outer (per-tile deadlines spaced 0.86us
    instead of 0.2us), rotating over 6 PSUM banks; superblock 3 runs
    ht-outer with the FC10 matmuls of superblocks 2/3 interleaved at its
    group boundaries so the closing ladder is short
  - relu + bias split ACT/DVE halves; logits leave per superblock
"""

import numpy as np
import ml_dtypes

B = 16384
IMG = 28
K = 3
OUT = IMG - K + 1  # 26
M26 = OUT * OUT  # 676
Q = IMG * IMG  # 784
HID = 512
NCLS = 10

NCORES = 8
BL = B // NCORES  # 2048 rows per core
QT = 112  # q-tile height (partition dim), 7 tiles
NQT = Q // QT  # 7
SB = 512  # batch superblock (matmul N)
NSB = BL // SB  # 4
NHT = HID // 128  # 4
NWARM = 26  # short dummy matmuls ramping the PE during the DMA prologue

TRACE = False  # set by test harness to capture an NTFF profile
_CACHE = {}

_BF16 = ml_dtypes.bfloat16


def _build():
    import concourse.bacc as bacc
    import concourse.mybir as mybir
    import concourse.tile as tile

    f32 = mybir.dt.float32
    bf16 = mybir.dt.bfloat16
    AF = mybir.ActivationFunctionType

    nc = bacc.Bacc("TRN2", target_bir_lowering=False, debug=False)

    xt_d = nc.dram_tensor("xt", [Q, BL], bf16, kind="ExternalInput")
    w1_d = nc.dram_tensor("w1e", [QT, NQT * HID], bf16, kind="ExternalInput")
    b1_d = nc.dram_tensor("b1l", [128, NHT], f32, kind="ExternalInput")
    w2_d = nc.dram_tensor("w2l", [128, NHT * NCLS], bf16, kind="ExternalInput")
    b2_d = nc.dram_tensor("b2l", [NCLS, 1], f32, kind="ExternalInput")
    out_d = nc.dram_tensor("out", [NCLS, BL], f32, kind="ExternalOutput")

    # L1 psum bank rotation: superblock s uses banks (4s+ht) mod 6, so the
    # next superblock's first two groups land on banks the previous one
    # never touched and the rest reuse banks whose relu fired long before.
    LBANK = [[(4 * s + ht) % 6 for ht in range(NHT)] for s in range(NSB)]

    with tile.TileContext(nc) as tc:
        with (
            tc.tile_pool(name="weights", bufs=1) as wp,
            tc.tile_pool(name="xin", bufs=1) as xp,
            tc.tile_pool(name="hid", bufs=1) as hp,
            tc.tile_pool(name="lgts", bufs=1) as lp,
            tc.tile_pool(name="ps", bufs=1, space="PSUM") as pp,
        ):
            # ---- PE warmup on scratch data: short matmuls so real work
            # cuts in with little overshoot once its inputs land.
            scratch = wp.tile([128, 128], bf16, tag="scratch")
            nc.gpsimd.memset(scratch[:], 0.0)
            warm = pp.tile([128, SB], f32, tag="L4", name="warm")
            for _ in range(NWARM):
                nc.tensor.matmul(
                    warm[:, :128],
                    lhsT=scratch[:],
                    rhs=scratch[:],
                    start=True,
                    stop=True,
                )

            # ---- supply.  The scalar (ACT) engine must stay free for relu
            # work from ~17us on, so it only carries the two w1e[0] halves;
            # every other transfer alternates sync/gpsimd in consumption
            # order.  The t0 pair is split in halves for an earlier first
            # arrival.
            HSB = SB // 2
            rings = [nc.sync, nc.gpsimd]
            ri = {"i": 0}

            def ring():
                r = rings[ri["i"] % 2]
                ri["i"] += 1
                return r

            w1e, xs = [], {}

            def trig_x(s, t, r=None):
                xx = xp.tile([QT, SB], bf16, tag=f"x{s}_{t}")
                (r or ring()).dma_start(
                    out=xx[:],
                    in_=xt_d[QT * t : QT * (t + 1), SB * s : SB * (s + 1)],
                )
                xs[(s, t)] = xx

            for t in range(NQT):
                we = wp.tile([QT, HID], bf16, tag=f"we{t}")
                if t == 0:
                    nc.scalar.dma_start(out=we[:], in_=w1_d[:, 0:HID])
                else:
                    ring().dma_start(
                        out=we[:], in_=w1_d[:, HID * t : HID * (t + 1)]
                    )
                w1e.append(we)
                if t == 0:
                    # two column-half transfers: the first L1 matmuls need
                    # only the left half, landing ~0.7us earlier
                    xx = xp.tile([QT, SB], bf16, tag="x0_0")
                    nc.sync.dma_start(out=xx[:, :HSB], in_=xt_d[0:QT, 0:HSB])
                    nc.sync.dma_start(out=xx[:, HSB:], in_=xt_d[0:QT, HSB:SB])
                    xs[(0, 0)] = xx
                else:
                    trig_x(0, t)

            b1 = wp.tile([128, NHT], f32, tag="b1")
            ring().dma_start(out=b1[:], in_=b1_d[:, :])
            for t in range(NQT):
                trig_x(1, t)
            w2 = wp.tile([128, NHT * NCLS], bf16, tag="w2")
            ring().dma_start(out=w2[:], in_=w2_d[:, :])
            b2 = wp.tile([NCLS, 1], f32, tag="b2")
            ring().dma_start(out=b2[:], in_=b2_d[:, :])
            # superblocks 2-3 are gated behind x(0,0)'s arrival: a dummy
            # write into each tile's pool slot reads x(0,0), so the real
            # DMA (slot WAW) can only start once the stream's first tile
            # has landed -- the early window's bandwidth goes to the tiles
            # the PE needs first.  Deadlines have ~6us of slack.
            for s in range(2, NSB):
                for t in range(NQT):
                    dmy = xp.tile(
                        [QT, SB], bf16, tag=f"x{s}_{t}", name=f"gate{s}_{t}"
                    )
                    nc.vector.tensor_add(
                        dmy[0:1, 0:4], xs[(0, 0)][0:1, 0:4], xs[(0, 0)][0:1, 0:4]
                    )
            for s in range(2, NSB):
                for t in range(NQT):
                    trig_x(s, t)

            hs_all = {}
            lg = lp.tile([NCLS, BL], f32, tag="lg")
            half = SB // 2

            def relu(s, ht, ps):
                h = hp.tile([128, SB], bf16, tag=f"h{s}_{ht}")
                nc.scalar.activation(
                    h[:, :half],
                    ps[:, :half],
                    AF.Relu,
                    bias=b1[:, ht : ht + 1],
                    scale=1.0,
                )
                nc.vector.tensor_scalar(
                    h[:, half:],
                    ps[:, half:],
                    b1[:, ht : ht + 1],
                    0.0,
                    mybir.AluOpType.add,
                    mybir.AluOpType.max,
                )
                hs_all[(s, ht)] = h

            def l1_block(s):
                # t-outer: the four ht accumulation groups advance together
                # so each x tile is consumed once per 0.86us, tracking DMA
                # arrival order with slack.
                pss = [
                    pp.tile([128, SB], f32, tag=f"L{LBANK[s][ht]}", name=f"ps{s}_{ht}")
                    for ht in range(NHT)
                ]
                for t in range(NQT):
                    for ht in range(NHT):
                        if s == 0 and t == 0:
                            # column-halved so the stream starts on the
                            # first half-transfer of x(0,0)
                            nc.tensor.matmul(
                                pss[ht][:, :half],
                                lhsT=w1e[0][:, 128 * ht : 128 * (ht + 1)],
                                rhs=xs[(0, 0)][:, :half],
                                start=True,
                                stop=False,
                            )
                            nc.tensor.matmul(
                                pss[ht][:, half:],
                                lhsT=w1e[0][:, 128 * ht : 128 * (ht + 1)],
                                rhs=xs[(0, 0)][:, half:],
                                start=False,
                                stop=False,
                                skip_group_check=True,
                            )
                        else:
                            nc.tensor.matmul(
                                pss[ht][:],
                                lhsT=w1e[t][:, 128 * ht : 128 * (ht + 1)],
                                rhs=xs[(s, t)][:],
                                start=(t == 0),
                                stop=(t == NQT - 1),
                            )
                for ht in range(NHT):
                    relu(s, ht, pss[ht])

            l2ps = {}

            def l2_mm(s, ht):
                if ht == 0:
                    l2ps[s] = pp.tile(
                        [NCLS, SB], f32, tag=f"O{s % 2}", name=f"ps2_{s}"
                    )
                nc.tensor.matmul(
                    l2ps[s][:],
                    lhsT=w2[:, NCLS * ht : NCLS * (ht + 1)],
                    rhs=hs_all[(s, ht)][:],
                    start=(ht == 0),
                    stop=(ht == NHT - 1),
                )

            def l2_bias_out(s, last=False):
                ps2 = l2ps[s]
                lo = SB * s
                nc.vector.tensor_scalar(
                    lg[:, lo : lo + half],
                    ps2[:, :half],
                    b2[:, 0:1],
                    None,
                    mybir.AluOpType.add,
                )
                if last:
                    # DVE observes the closing matmul's semaphore ~0.5us
                    # faster than ACT, so the final bias stays on DVE, and
                    # the logits leave in one sync-ring transfer
                    nc.vector.tensor_scalar(
                        lg[:, lo + half : lo + SB],
                        ps2[:, half:],
                        b2[:, 0:1],
                        None,
                        mybir.AluOpType.add,
                    )
                    nc.sync.dma_start(
                        out=out_d[:, lo : lo + SB], in_=lg[:, lo : lo + SB]
                    )
                else:
                    nc.scalar.activation(
                        lg[:, lo + half : lo + SB],
                        ps2[:, half:],
                        AF.Identity,
                        bias=b2[:, 0:1],
                        scale=1.0,
                    )
                    eng = nc.sync if s % 2 == 0 else nc.gpsimd
                    eng.dma_start(out=out_d[:, lo : lo + SB], in_=lg[:, lo : lo + SB])

            def l2_block(s):
                for ht in range(NHT):
                    l2_mm(s, ht)
                l2_bias_out(s)

            def l1_block_last(s):
                # ht-outer with the two pending FC10 blocks interleaved at
                # group boundaries.  The last ht runs as two column-half
                # accumulation groups so its left-half relu (and the
                # matching FC10 half-matmul) completes while the right half
                # still streams, shortening the closing ladder.
                pss = [
                    pp.tile([128, SB], f32, tag=f"L{LBANK[s][ht]}", name=f"ps{s}_{ht}")
                    for ht in range(NHT)
                ]
                hlast = hp.tile([128, SB], bf16, tag=f"h{s}_{NHT - 1}")
                hs_all[(s, NHT - 1)] = hlast
                for ht in range(NHT - 1):
                    for t in range(NQT):
                        nc.tensor.matmul(
                            pss[ht][:],
                            lhsT=w1e[t][:, 128 * ht : 128 * (ht + 1)],
                            rhs=xs[(s, t)][:],
                            start=(t == 0),
                            stop=(t == NQT - 1),
                        )
                    relu(s, ht, pss[ht])
                    l2_mm(s - 1, ht)
                    if ht >= 1:
                        l2_mm(s, ht - 1)
                ht = NHT - 1
                for t in range(NQT):  # left half of the last ht
                    nc.tensor.matmul(
                        pss[ht][:, :half],
                        lhsT=w1e[t][:, 128 * ht : 128 * (ht + 1)],
                        rhs=xs[(s, t)][:, :half],
                        start=(t == 0),
                        stop=(t == NQT - 1),
                    )
                nc.scalar.activation(
                    hlast[:, :half],
                    pss[ht][:, :half],
                    AF.Relu,
                    bias=b1[:, ht : ht + 1],
                    scale=1.0,
                )
                l2_mm(s - 1, ht)
                # right half lands in a free bank (L5, idle since its s2
                # relu) so its group-start zeroing can never touch the
                # left half's bank
                pssR = pp.tile([128, SB], f32, tag="L5", name=f"ps{s}_{ht}R")
                for t in range(NQT):
                    nc.tensor.matmul(
                        pssR[:, half:],
                        lhsT=w1e[t][:, 128 * ht : 128 * (ht + 1)],
                        rhs=xs[(s, t)][:, half:],
                        start=(t == 0),
                        stop=(t == NQT - 1),
                    )
                nc.vector.tensor_scalar(
                    hlast[:, half:],
                    pssR[:, half:],
                    b1[:, ht : ht + 1],
                    0.0,
                    mybir.AluOpType.add,
                    mybir.AluOpType.max,
                )
                l2_mm(s, ht - 1)
                l2_bias_out(s - 1)
                # last FC10 contribution in two column halves: the left one
                # only needs the left relu, which fired half a group ago
                nc.tensor.matmul(
                    l2ps[s][:, :half],
                    lhsT=w2[:, NCLS * ht : NCLS * (ht + 1)],
                    rhs=hlast[:, :half],
                    start=False,
                    stop=True,
                    skip_group_check=True,
                )
                nc.tensor.matmul(
                    l2ps[s][:, half:],
                    lhsT=w2[:, NCLS * ht : NCLS * (ht + 1)],
                    rhs=hlast[:, half:],
                    start=False,
                    stop=True,
                    skip_group_check=True,
                )
                l2_bias_out(s, last=True)

            l1_block(0)
            l1_block(1)
            l2_block(0)
            l1_block(2)
            l2_block(1)
            l1_block_last(3)

    nc.compile()
    return nc


def _get_nc():
    if "nc" not in _CACHE:
        _CACHE["nc"] = _build()
    return _CACHE["nc"]


def kernel(x, conv_w, W1, b1, W2, b2):
    from concourse.bass_utils import run_bass_kernel_spmd

    nc = _get_nc()

    # Host fold: W1_eff[q] = sum_{di,dj} conv_w[di,dj] * W1[m(q,di,dj)]
    # (banded C @ W1 without materializing C).
    w1f = np.asarray(W1, np.float32)
    cw = np.asarray(conv_w, np.float32)
    W1e = np.zeros((Q, HID), dtype=np.float32)
    ii, jj = np.meshgrid(np.arange(OUT), np.arange(OUT), indexing="ij")
    m = (OUT * ii + jj).ravel()
    for di in range(K):
        for dj in range(K):
            q = ((ii + di) * IMG + (jj + dj)).ravel()
            np.add.at(W1e, q, cw[di, dj] * w1f[m])
    # chunk layout: w1e[p, HID*t : HID*(t+1)] = W1e[QT*t + p, :]
    w1p = np.ascontiguousarray(
        W1e.reshape(NQT, QT, HID).transpose(1, 0, 2).reshape(QT, NQT * HID)
    ).astype(_BF16)

    b1l = np.ascontiguousarray(
        np.asarray(b1, np.float32).reshape(NHT, 128).T
    )  # [128, 4]
    w2l = np.ascontiguousarray(
        np.asarray(W2, np.float32)
        .reshape(NHT, 128, NCLS)
        .transpose(1, 0, 2)
        .reshape(128, NHT * NCLS)
    ).astype(_BF16)
    b2l = np.asarray(b2, np.float32).reshape(NCLS, 1)

    xf = np.asarray(x, np.float32)
    in_maps = []
    for c in range(NCORES):
        xt = np.ascontiguousarray(xf[c * BL : (c + 1) * BL].T).astype(_BF16)
        in_maps.append(
            {
                "xt": xt,
                "w1e": w1p,
                "b1l": b1l,
                "w2l": w2l,
                "b2l": b2l,
            }
        )

    kwargs = {}
    if TRACE:
        import profhook  # noqa: F401  (installs the NTFF hook shim)
        import tempfile

        kwargs = {"trace": True, "tmpdir": tempfile.mkdtemp(prefix="ntff_")}
    res = run_bass_kernel_spmd(nc, in_maps, core_ids=list(range(NCORES)), **kwargs)
    if TRACE:
        _CACHE["last_results"] = res

    out = np.concatenate(
        [np.ascontiguousarray(res.results[c]["out"].T) for c in range(NCORES)], axis=0
    ).astype(np.float32)
    return out


# revision 25
# speedup vs baseline: 1.0185x; 1.0185x over previous
"""Trainium2 Bass kernel for DigitConvolutionalModel (conv3x3 -> FC512 -> FC10).

Math: the 3x3 valid conv is linear, so y_flat = x @ C with C [784, 676]
banded; W1_eff = C @ W1 is folded on the HOST (9 terms per row), so the
device runs only
    logits = relu(x @ W1_eff + b1) @ W2 + b2
Data-parallel across 8 cores on the batch dim (2048 rows per core).

Device schedule per core:
  - warmup matmuls ramp the PE while the first chunks stream in
  - supply (w1eff chunks, x per (superblock, t)) triggered round-robin
    over the three DMA rings in exact consumption order; per-transfer
    completion latency is ~1.5-3.5us under load, aggregate ~240GB/s
  - L1 superblocks 0-2 run t-outer (per-tile deadlines spaced 0.86us
    instead of 0.2us), rotating over 6 PSUM banks; superblock 3 runs
    ht-outer with the FC10 matmuls of superblocks 2/3 interleaved at its
    group boundaries so the closing ladder is short
  - relu + bias split ACT/DVE halves; logits leave per superblock
"""

import numpy as np
import ml_dtypes

B = 16384
IMG = 28
K = 3
OUT = IMG - K + 1  # 26
M26 = OUT * OUT  # 676
Q = IMG * IMG  # 784
HID = 512
NCLS = 10

NCORES = 8
BL = B // NCORES  # 2048 rows per core
QT = 112  # q-tile height (partition dim), 7 tiles
NQT = Q // QT  # 7
SB = 512  # batch superblock (matmul N)
NSB = BL // SB  # 4
NHT = HID // 128  # 4
NWARM = 26  # short dummy matmuls ramping the PE during the DMA prologue

TRACE = False  # set by test harness to capture an NTFF profile
_CACHE = {}

_BF16 = ml_dtypes.bfloat16


def _build():
    import concourse.bacc as bacc
    import concourse.mybir as mybir
    import concourse.tile as tile

    f32 = mybir.dt.float32
    bf16 = mybir.dt.bfloat16
    AF = mybir.ActivationFunctionType

    nc = bacc.Bacc("TRN2", target_bir_lowering=False, debug=False)

    xt_d = nc.dram_tensor("xt", [Q, BL], bf16, kind="ExternalInput")
    w1_d = nc.dram_tensor("w1e", [QT, NQT * HID], bf16, kind="ExternalInput")
    b1_d = nc.dram_tensor("b1l", [128, NHT], f32, kind="ExternalInput")
    w2_d = nc.dram_tensor("w2l", [128, NHT * NCLS], bf16, kind="ExternalInput")
    b2_d = nc.dram_tensor("b2l", [NCLS, 1], f32, kind="ExternalInput")
    out_d = nc.dram_tensor("out", [NCLS, BL], f32, kind="ExternalOutput")

    # L1 psum bank rotation: superblock s uses banks (4s+ht) mod 6, so the
    # next superblock's first two groups land on banks the previous one
    # never touched and the rest reuse banks whose relu fired long before.
    LBANK = [[(4 * s + ht) % 6 for ht in range(NHT)] for s in range(NSB)]

    with tile.TileContext(nc) as tc:
        with (
            tc.tile_pool(name="weights", bufs=1) as wp,
            tc.tile_pool(name="xin", bufs=1) as xp,
            tc.tile_pool(name="hid", bufs=1) as hp,
            tc.tile_pool(name="lgts", bufs=1) as lp,
            tc.tile_pool(name="ps", bufs=1, space="PSUM") as pp,
        ):
            # ---- PE warmup on scratch data: short matmuls so real work
            # cuts in with little overshoot once its inputs land.
            scratch = wp.tile([128, 128], bf16, tag="scratch")
            nc.gpsimd.memset(scratch[:], 0.0)
            warm = pp.tile([128, SB], f32, tag="L4", name="warm")
            for _ in range(NWARM):
                nc.tensor.matmul(
                    warm[:, :128],
                    lhsT=scratch[:],
                    rhs=scratch[:],
                    start=True,
                    stop=True,
                )

            # ---- supply.  The scalar (ACT) engine must stay free for relu
            # work from ~17us on, so it only carries the two w1e[0] halves;
            # every other transfer alternates sync/gpsimd in consumption
            # order.  The t0 pair is split in halves for an earlier first
            # arrival.
            HSB = SB // 2
            rings = [nc.sync, nc.gpsimd]
            ri = {"i": 0}

            def ring():
                r = rings[ri["i"] % 2]
                ri["i"] += 1
                return r

            w1e, xs = [], {}

            def trig_x(s, t, r=None):
                xx = xp.tile([QT, SB], bf16, tag=f"x{s}_{t}")
                (r or ring()).dma_start(
                    out=xx[:],
                    in_=xt_d[QT * t : QT * (t + 1), SB * s : SB * (s + 1)],
                )
                xs[(s, t)] = xx

            for t in range(NQT):
                we = wp.tile([QT, HID], bf16, tag=f"we{t}")
                if t == 0:
                    nc.scalar.dma_start(out=we[:], in_=w1_d[:, 0:HID])
                else:
                    ring().dma_start(
                        out=we[:], in_=w1_d[:, HID * t : HID * (t + 1)]
                    )
                w1e.append(we)
                if t == 0:
                    # two column-half transfers: the first L1 matmuls need
                    # only the left half, landing ~0.7us earlier
                    xx = xp.tile([QT, SB], bf16, tag="x0_0")
                    nc.sync.dma_start(out=xx[:, :HSB], in_=xt_d[0:QT, 0:HSB])
                    nc.sync.dma_start(out=xx[:, HSB:], in_=xt_d[0:QT, HSB:SB])
                    xs[(0, 0)] = xx
                else:
                    trig_x(0, t)

            b1 = wp.tile([128, NHT], f32, tag="b1")
            ring().dma_start(out=b1[:], in_=b1_d[:, :])
            for t in range(NQT):
                trig_x(1, t)
            w2 = wp.tile([128, NHT * NCLS], bf16, tag="w2")
            ring().dma_start(out=w2[:], in_=w2_d[:, :])
            b2 = wp.tile([NCLS, 1], f32, tag="b2")
            ring().dma_start(out=b2[:], in_=b2_d[:, :])
            # superblocks 2-3 are gated behind the LAST s1 tile's arrival:
            # a dummy write into each tile's pool slot reads x(1, t6), so
            # the real DMA (slot WAW) can only start once every tile the
            # stream needs first has landed -- the early window's bandwidth
            # goes entirely to s0/s1.  Deadlines keep ~6us of slack.
            gate_src = xs[(1, NQT - 1)]
            for s in range(2, NSB):
                for t in range(NQT):
                    dmy = xp.tile(
                        [QT, SB], bf16, tag=f"x{s}_{t}", name=f"gate{s}_{t}"
                    )
                    nc.vector.tensor_add(
                        dmy[0:1, 0:4], gate_src[0:1, 0:4], gate_src[0:1, 0:4]
                    )
            for s in range(2, NSB):
                for t in range(NQT):
                    trig_x(s, t)

            hs_all = {}
            lg = lp.tile([NCLS, BL], f32, tag="lg")
            half = SB // 2

            def relu(s, ht, ps):
                h = hp.tile([128, SB], bf16, tag=f"h{s}_{ht}")
                nc.scalar.activation(
                    h[:, :half],
                    ps[:, :half],
                    AF.Relu,
                    bias=b1[:, ht : ht + 1],
                    scale=1.0,
                )
                nc.vector.tensor_scalar(
                    h[:, half:],
                    ps[:, half:],
                    b1[:, ht : ht + 1],
                    0.0,
                    mybir.AluOpType.add,
                    mybir.AluOpType.max,
                )
                hs_all[(s, ht)] = h

            def l1_block(s):
                # t-outer: the four ht accumulation groups advance together
                # so each x tile is consumed once per 0.86us, tracking DMA
                # arrival order with slack.
                pss = [
                    pp.tile([128, SB], f32, tag=f"L{LBANK[s][ht]}", name=f"ps{s}_{ht}")
                    for ht in range(NHT)
                ]
                for t in range(NQT):
                    for ht in range(NHT):
                        if s == 0 and t == 0:
                            # column-halved so the stream starts on the
                            # first half-transfer of x(0,0)
                            nc.tensor.matmul(
                                pss[ht][:, :half],
                                lhsT=w1e[0][:, 128 * ht : 128 * (ht + 1)],
                                rhs=xs[(0, 0)][:, :half],
                                start=True,
                                stop=False,
                            )
                            nc.tensor.matmul(
                                pss[ht][:, half:],
                                lhsT=w1e[0][:, 128 * ht : 128 * (ht + 1)],
                                rhs=xs[(0, 0)][:, half:],
                                start=False,
                                stop=False,
                                skip_group_check=True,
                            )
                        else:
                            nc.tensor.matmul(
                                pss[ht][:],
                                lhsT=w1e[t][:, 128 * ht : 128 * (ht + 1)],
                                rhs=xs[(s, t)][:],
                                start=(t == 0),
                                stop=(t == NQT - 1),
                            )
                for ht in range(NHT):
                    relu(s, ht, pss[ht])

            l2ps = {}

            def l2_mm(s, ht):
                if ht == 0:
                    l2ps[s] = pp.tile(
                        [NCLS, SB], f32, tag=f"O{s % 2}", name=f"ps2_{s}"
                    )
                nc.tensor.matmul(
                    l2ps[s][:],
                    lhsT=w2[:, NCLS * ht : NCLS * (ht + 1)],
                    rhs=hs_all[(s, ht)][:],
                    start=(ht == 0),
                    stop=(ht == NHT - 1),
                )

            def l2_bias_out(s, last=False):
                ps2 = l2ps[s]
                lo = SB * s
                nc.vector.tensor_scalar(
                    lg[:, lo : lo + half],
                    ps2[:, :half],
                    b2[:, 0:1],
                    None,
                    mybir.AluOpType.add,
                )
                if last:
                    # DVE observes the closing matmul's semaphore ~0.5us
                    # faster than ACT, so the final bias stays on DVE, and
                    # the logits leave in one sync-ring transfer
                    nc.vector.tensor_scalar(
                        lg[:, lo + half : lo + SB],
                        ps2[:, half:],
                        b2[:, 0:1],
                        None,
                        mybir.AluOpType.add,
                    )
                    nc.sync.dma_start(
                        out=out_d[:, lo : lo + SB], in_=lg[:, lo : lo + SB]
                    )
                else:
                    nc.scalar.activation(
                        lg[:, lo + half : lo + SB],
                        ps2[:, half:],
                        AF.Identity,
                        bias=b2[:, 0:1],
                        scale=1.0,
                    )
                    eng = nc.sync if s % 2 == 0 else nc.gpsimd
                    eng.dma_start(out=out_d[:, lo : lo + SB], in_=lg[:, lo : lo + SB])

            def l2_block(s):
                for ht in range(NHT):
                    l2_mm(s, ht)
                l2_bias_out(s)

            def l1_block_last(s):
                # ht-outer with the two pending FC10 blocks interleaved at
                # group boundaries.  The last ht runs as two column-half
                # accumulation groups so its left-half relu (and the
                # matching FC10 half-matmul) completes while the right half
                # still streams, shortening the closing ladder.
                pss = [
                    pp.tile([128, SB], f32, tag=f"L{LBANK[s][ht]}", name=f"ps{s}_{ht}")
                    for ht in range(NHT)
                ]
                hlast = hp.tile([128, SB], bf16, tag=f"h{s}_{NHT - 1}")
                hs_all[(s, NHT - 1)] = hlast
                for ht in range(NHT - 1):
                    for t in range(NQT):
                        nc.tensor.matmul(
                            pss[ht][:],
                            lhsT=w1e[t][:, 128 * ht : 128 * (ht + 1)],
                            rhs=xs[(s, t)][:],
                            start=(t == 0),
                            stop=(t == NQT - 1),
                        )
                    relu(s, ht, pss[ht])
                    l2_mm(s - 1, ht)
                    if ht >= 1:
                        l2_mm(s, ht - 1)
                ht = NHT - 1
                for t in range(NQT):  # left half of the last ht
                    nc.tensor.matmul(
                        pss[ht][:, :half],
                        lhsT=w1e[t][:, 128 * ht : 128 * (ht + 1)],
                        rhs=xs[(s, t)][:, :half],
                        start=(t == 0),
                        stop=(t == NQT - 1),
                    )
                nc.scalar.activation(
                    hlast[:, :half],
                    pss[ht][:, :half],
                    AF.Relu,
                    bias=b1[:, ht : ht + 1],
                    scale=1.0,
                )
                l2_mm(s - 1, ht)
                # right half lands in a free bank (L5, idle since its s2
                # relu) so its group-start zeroing can never touch the
                # left half's bank
                pssR = pp.tile([128, SB], f32, tag="L5", name=f"ps{s}_{ht}R")
                for t in range(NQT):
                    nc.tensor.matmul(
                        pssR[:, half:],
                        lhsT=w1e[t][:, 128 * ht : 128 * (ht + 1)],
                        rhs=xs[(s, t)][:, half:],
                        start=(t == 0),
                        stop=(t == NQT - 1),
                    )
                nc.vector.tensor_scalar(
                    hlast[:, half:],
                    pssR[:, half:],
                    b1[:, ht : ht + 1],
                    0.0,
                    mybir.AluOpType.add,
                    mybir.AluOpType.max,
                )
                l2_mm(s, ht - 1)
                l2_bias_out(s - 1)
                # last FC10 contribution in two column halves: the left one
                # only needs the left relu, which fired half a group ago
                nc.tensor.matmul(
                    l2ps[s][:, :half],
                    lhsT=w2[:, NCLS * ht : NCLS * (ht + 1)],
                    rhs=hlast[:, :half],
                    start=False,
                    stop=True,
                    skip_group_check=True,
                )
                nc.tensor.matmul(
                    l2ps[s][:, half:],
                    lhsT=w2[:, NCLS * ht : NCLS * (ht + 1)],
                    rhs=hlast[:, half:],
                    start=False,
                    stop=True,
                    skip_group_check=True,
                )
                l2_bias_out(s, last=True)

            l1_block(0)
            l1_block(1)
            l2_block(0)
            l1_block(2)
            l2_block(1)
            l1_block_last(3)

    nc.compile()
    return nc


def _get_nc():
    if "nc" not in _CACHE:
        _CACHE["nc"] = _build()
    return _CACHE["nc"]


def kernel(x, conv_w, W1, b1, W2, b2):
    from concourse.bass_utils import run_bass_kernel_spmd

    nc = _get_nc()

    # Host fold: W1_eff[q] = sum_{di,dj} conv_w[di,dj] * W1[m(q,di,dj)]
    # (banded C @ W1 without materializing C).
    w1f = np.asarray(W1, np.float32)
    cw = np.asarray(conv_w, np.float32)
    W1e = np.zeros((Q, HID), dtype=np.float32)
    ii, jj = np.meshgrid(np.arange(OUT), np.arange(OUT), indexing="ij")
    m = (OUT * ii + jj).ravel()
    for di in range(K):
        for dj in range(K):
            q = ((ii + di) * IMG + (jj + dj)).ravel()
            np.add.at(W1e, q, cw[di, dj] * w1f[m])
    # chunk layout: w1e[p, HID*t : HID*(t+1)] = W1e[QT*t + p, :]
    w1p = np.ascontiguousarray(
        W1e.reshape(NQT, QT, HID).transpose(1, 0, 2).reshape(QT, NQT * HID)
    ).astype(_BF16)

    b1l = np.ascontiguousarray(
        np.asarray(b1, np.float32).reshape(NHT, 128).T
    )  # [128, 4]
    w2l = np.ascontiguousarray(
        np.asarray(W2, np.float32)
        .reshape(NHT, 128, NCLS)
        .transpose(1, 0, 2)
        .reshape(128, NHT * NCLS)
    ).astype(_BF16)
    b2l = np.asarray(b2, np.float32).reshape(NCLS, 1)

    xf = np.asarray(x, np.float32)
    in_maps = []
    for c in range(NCORES):
        xt = np.ascontiguousarray(xf[c * BL : (c + 1) * BL].T).astype(_BF16)
        in_maps.append(
            {
                "xt": xt,
                "w1e": w1p,
                "b1l": b1l,
                "w2l": w2l,
                "b2l": b2l,
            }
        )

    kwargs = {}
    if TRACE:
        import profhook  # noqa: F401  (installs the NTFF hook shim)
        import tempfile

        kwargs = {"trace": True, "tmpdir": tempfile.mkdtemp(prefix="ntff_")}
    res = run_bass_kernel_spmd(nc, in_maps, core_ids=list(range(NCORES)), **kwargs)
    if TRACE:
        _CACHE["last_results"] = res

    out = np.concatenate(
        [np.ascontiguousarray(res.results[c]["out"].T) for c in range(NCORES)], axis=0
    ).astype(np.float32)
    return out
